# revision 24
# baseline (speedup 1.0000x reference)
"""Dense transformer block (RMSNorm+MHA+residual, RMSNorm+SwiGLU+residual)
on 8 trn2 NeuronCores. Sharding: 2 cores per batch element; each core
computes the block output for 1024 of its batch's 2048 tokens, redundantly
computing K/V for the full sequence (attention keys are permutation
invariant, so each core's xT puts its own 1024 query tokens first).
No inter-core communication.

All on-chip tensors are feature-major ([feature, token]) so every matmul
contraction lands on the partition dim. Softmax denominators come free
from a ones-column appended to V.

v2: fp8 DoubleRow FFN (2x tensor throughput), softmax exp batched over
4 PSUM banks per ACTIVATE (amortizes the per-instruction overhead),
silu via tanh / rsqrt via ln+exp (keeps the scalar engine mostly on one
activation table set), gpsimd partition-broadcast instead of PSUM
broadcast matmuls, batched strided DMAs, and token-sliced emission so
FFN matmuls overlap the exp-bound attention window.
"""
import sys
from contextlib import ExitStack

import numpy as np

sys.path.insert(0, "/opt/trn_rl_repo")

import ml_dtypes  # noqa: E402
import concourse.bass as bass  # noqa: E402
from concourse import bacc  # noqa: E402
import concourse.tile as tile  # noqa: E402
from concourse import mybir  # noqa: E402
from concourse import bass_utils  # noqa: E402

P = 128
D = 1024          # d_model
L = 2048          # full seq per core (keys)
LQ = 1024         # query tokens per core
NH = 16
HD = 64
HID = 4096
EPS = 1e-6
NDT = D // P      # 8 feature tiles
NKT = L // P      # 16 key tiles
NHT = HID // P    # 32 hidden tiles
G = 2             # kt blocks per exp group (2 PSUM banks)
SW = 64.0         # fp8 weight scale (wg8/wh8/wo8 = 64*w)
SGH = 16.0        # ghT8 = 16 * silu(g) * h
F32 = mybir.dt.float32
BF16 = mybir.dt.bfloat16
FP8 = mybir.dt.float8e4
F32R = mybir.dt.float32r
AF = mybir.ActivationFunctionType
ALU = mybir.AluOpType
DR = mybir.MatmulPerfMode.DoubleRow

SIM_TIME_NS = None


def build_nc():
    global SIM_TIME_NS
    nc = bacc.Bacc(None, target_bir_lowering=False)
    d = {}
    d["xT"] = nc.dram_tensor("xT", [D, L], F32, kind="ExternalInput")
    d["wqT"] = nc.dram_tensor("wqT", [D, D], BF16, kind="ExternalInput")
    d["wkT"] = nc.dram_tensor("wkT", [D, D], BF16, kind="ExternalInput")
    d["wvT"] = nc.dram_tensor("wvT", [D, D], BF16, kind="ExternalInput")
    d["woT"] = nc.dram_tensor("woT", [D, D], BF16, kind="ExternalInput")
    d["bo_t"] = nc.dram_tensor("bo_t", [P, NDT], F32, kind="ExternalInput")
    d["wgT"] = nc.dram_tensor("wgT", [D, HID], F32R, kind="ExternalInput")
    d["whT"] = nc.dram_tensor("whT", [D, HID], F32R, kind="ExternalInput")
    d["woB"] = nc.dram_tensor("woB", [HID, D], F32R, kind="ExternalInput")
    d["bout_t"] = nc.dram_tensor("bout_t", [P, NDT], F32, kind="ExternalInput")
    d["yT"] = nc.dram_tensor("yT", [D, LQ], F32, kind="ExternalOutput")

    with tile.TileContext(nc) as tc:
        _body(tc, nc, d)
        _, snap = tc.schedule_and_allocate()
        SIM_TIME_NS = snap.time
    nc.compile()
    return nc


def _body(tc, nc, d):
    xTr = d["xT"].rearrange("(dt p) l -> p dt l", p=P)
    with ExitStack() as pp_ctx:
        pp = pp_ctx.enter_context(tc.tile_pool(name="persist", bufs=1))
        bo_sb = pp.tile([P, NDT], F32, tag="bo")
        bout_sb = pp.tile([P, NDT], F32, tag="bout")
        ones_col = pp.tile([P, 1], BF16, tag="ones")
        eps_t = pp.tile([1, 1], F32, tag="eps")
        x1T = pp.tile([P, NDT, LQ], F32, tag="x1T")
        x1np_o = pp_ctx.enter_context(tc.tile_pool(name="x1n", bufs=2))
        # gate/hidden/misc PSUM pools span attention (Wo/ss2) and FFN
        gpsp = pp_ctx.enter_context(tc.tile_pool(name="gps", bufs=1, space="PSUM"))
        hpsp = pp_ctx.enter_context(tc.tile_pool(name="hps", bufs=1, space="PSUM"))
        mpsp = pp_ctx.enter_context(tc.tile_pool(name="mps", bufs=1, space="PSUM"))
        nc.sync.dma_start(out=bo_sb, in_=d["bo_t"][:, :])
        nc.sync.dma_start(out=bout_sb, in_=d["bout_t"][:, :])
        nc.vector.memset(ones_col, 1.0)
        nc.vector.memset(eps_t, EPS)
        x1ns = []

        # kqv pool closes after attention so FFN pools reuse its SBUF
        with tc.tile_pool(name="kqv", bufs=1) as kqvp:
            kT = kqvp.tile([P, NDT, L], BF16, tag="kT")
            qT = kqvp.tile([P, NDT, LQ], BF16, tag="qT")
            vt = kqvp.tile([P, NKT, NH, HD + 1], BF16, tag="vt")
            nc.vector.memset(vt[:, :, :, HD:HD + 1], 1.0)

            # ---- phase 0/1: load x, rmsnorm in place, project K/Q/V ----
            with tc.tile_pool(name="xt", bufs=1) as xtp:
                xt = xtp.tile([P, NDT, L], BF16, tag="xt")
                nc.gpsimd.dma_start(out=xt[:, 0:4, :], in_=xTr[:, 0:4, :])
                nc.gpsimd.dma_start(out=xt[:, 4:8, :], in_=xTr[:, 4:8, :])
                with tc.tile_pool(name="n1", bufs=3) as n1p, \
                     tc.tile_pool(name="bc1", bufs=1) as bc1p, \
                     tc.tile_pool(name="ss1", bufs=1, space="PSUM") as ssp:
                    ss_ps = ssp.tile([1, L], F32, tag="ss")
                    for dt_ in range(NDT):
                        for ls in range(L // 512):
                            sl = slice(ls * 512, (ls + 1) * 512)
                            sq = n1p.tile([P, 512], BF16, tag="sq")
                            nc.vector.tensor_mul(sq, xt[:, dt_, sl],
                                                 xt[:, dt_, sl])
                            nc.tensor.matmul(ss_ps[:, sl], ones_col, sq,
                                             start=(dt_ == 0),
                                             stop=(dt_ == NDT - 1))
                    bc = bc1p.tile([P, L], F32, tag="bc1")
                    for ls in range(L // 512):
                        sl = slice(ls * 512, (ls + 1) * 512)
                        lnr = bc1p.tile([1, 512], F32, tag="lnr")
                        nc.scalar.activation(lnr, ss_ps[:, sl], AF.Ln,
                                             bias=eps_t, scale=1.0 / D)
                        rr = bc1p.tile([1, 512], F32, tag="rr1")
                        nc.scalar.activation(rr, lnr, AF.Exp, scale=-0.5)
                        nc.gpsimd.partition_broadcast(bc[:, sl], rr)
                        for dt_ in range(NDT):
                            nc.vector.tensor_mul(
                                xt[:, dt_, sl], xt[:, dt_, sl], bc[:, sl])

                # K, Q projections (K first: scores depend on it)
                with tc.tile_pool(name="wblk", bufs=2) as wp, \
                     tc.tile_pool(name="wv", bufs=1) as wvp, \
                     tc.tile_pool(name="proj", bufs=4, space="PSUM") as prp:
                    for (w_d, dst, ntok) in ((d["wkT"], kT, L),
                                             (d["wqT"], qT, LQ)):
                        wr = w_d.rearrange("(dt p) f -> p dt f", p=P)
                        for ft in range(NDT):
                            wblk = wp.tile([P, NDT, P], BF16, tag="wblk")
                            nc.sync.dma_start(
                                out=wblk, in_=wr[:, :, ft * P:(ft + 1) * P])
                            for ns in range(ntok // 512):
                                sl = slice(ns * 512, (ns + 1) * 512)
                                ps = prp.tile([P, 512], F32, tag="pp")
                                for dt_ in range(NDT):
                                    nc.tensor.matmul(
                                        ps, wblk[:, dt_, :], xt[:, dt_, sl],
                                        start=(dt_ == 0),
                                        stop=(dt_ == NDT - 1))
                                nc.scalar.activation(dst[:, ft, sl], ps,
                                                     AF.Copy)
                    wvr = d["wvT"].rearrange("(dt p) f -> p dt f", p=P)
                    for hf in range(2):
                        wv = wvp.tile([P, NDT, 512], BF16, tag="wv")
                        nc.sync.dma_start(
                            out=wv, in_=wvr[:, :, hf * 512:(hf + 1) * 512])
                        for tt in range(NKT):
                            ps = prp.tile([P, 512], F32, tag="pp")
                            for dt_ in range(NDT):
                                nc.tensor.matmul(
                                    ps, xt[:, dt_, tt * P:(tt + 1) * P],
                                    wv[:, dt_, :],
                                    start=(dt_ == 0), stop=(dt_ == NDT - 1))
                            nc.vector.tensor_copy(
                                vt[:, tt, hf * 8:(hf + 1) * 8, 0:HD],
                                ps.rearrange("p (h e) -> p h e", h=8))

            # ---- phase 2: attention + Wo + norm2 per 512-token slice ----
            with ExitStack() as ectx:
                atp = ectx.enter_context(tc.tile_pool(name="attnp", bufs=1))
                ptp = ectx.enter_context(tc.tile_pool(name="pt", bufs=3))
                smp = ectx.enter_context(tc.tile_pool(name="sm", bufs=2))
                rbp = ectx.enter_context(tc.tile_pool(name="rb", bufs=2))
                xqp = ectx.enter_context(tc.tile_pool(name="xq", bufs=3))
                sqp = ectx.enter_context(tc.tile_pool(name="sq2p", bufs=2))
                wop = ectx.enter_context(tc.tile_pool(name="wo", bufs=3))
                stp = ectx.enter_context(
                    tc.tile_pool(name="st", bufs=2, space="PSUM"))
                accp = ectx.enter_context(
                    tc.tile_pool(name="acc", bufs=1, space="PSUM"))
                for ns in range(2):
                    qsl = slice(ns * 512, (ns + 1) * 512)
                    attnT = atp.tile([P, NDT, 512], BF16, tag="attnT")
                    for h in range(NH):
                        dt_ = h // 2
                        r0 = (h % 2) * HD
                        acc = accp.tile([HD + 1, 512], F32, tag="acc")
                        for g in range(NKT // G):
                            st = stp.tile([P, G, 512], F32, tag="st")
                            for j in range(G):
                                kt = g * G + j
                                nc.tensor.matmul(
                                    st[:, j, :],
                                    kT[r0:r0 + HD, dt_, kt * P:(kt + 1) * P],
                                    qT[r0:r0 + HD, dt_, qsl],
                                    start=True, stop=True)
                            pt = ptp.tile([P, G, 512], BF16, tag="pt")
                            nc.scalar.activation(pt, st, AF.Exp)
                            for j in range(G):
                                kt = g * G + j
                                nc.tensor.matmul(
                                    acc, vt[:, kt, h, :], pt[:, j, :],
                                    start=(kt == 0), stop=(kt == NKT - 1))
                        accS = smp.tile([HD + 1, 512], F32, tag="accS")
                        nc.vector.tensor_copy(accS, acc)
                        rrow = smp.tile([1, 512], F32, tag="row")
                        nc.vector.reciprocal(rrow, accS[HD:HD + 1, :])
                        rb = rbp.tile([HD, 512], F32, tag="rb")
                        nc.gpsimd.partition_broadcast(rb, rrow)
                        nc.vector.tensor_mul(
                            attnT[r0:r0 + HD, dt_, :], accS[0:HD, :], rb)

                    # Wo projection + residual -> x1T
                    wor = d["woT"].rearrange("(dt p) f -> p dt f", p=P)
                    for ft in range(NDT):
                        wblk = wop.tile([P, NDT, P], BF16, tag="woblk")
                        nc.sync.dma_start(
                            out=wblk, in_=wor[:, :, ft * P:(ft + 1) * P])
                        xq = xqp.tile([P, 512], F32, tag="xq")
                        nc.gpsimd.dma_start(out=xq, in_=xTr[:, ft, qsl])
                        ps = mpsp.tile([P, 512], F32, tag="m")
                        for fi in range(NDT):
                            nc.tensor.matmul(
                                ps, wblk[:, fi, :], attnT[:, fi, :],
                                start=(fi == 0), stop=(fi == NDT - 1))
                        nc.vector.scalar_tensor_tensor(
                            out=x1T[:, ft, qsl], in0=ps,
                            scalar=bo_sb[:, ft:ft + 1],
                            in1=xq, op0=ALU.add, op1=ALU.add)

                    # rmsnorm2 -> x1n (f32r for the f22 FFN)
                    ss2 = mpsp.tile([1, 512], F32, tag="m")
                    for dt_ in range(NDT):
                        sq2 = sqp.tile([P, 512], BF16, tag="sq2")
                        nc.vector.tensor_mul(
                            sq2, x1T[:, dt_, qsl], x1T[:, dt_, qsl])
                        nc.tensor.matmul(ss2, ones_col, sq2,
                                         start=(dt_ == 0),
                                         stop=(dt_ == NDT - 1))
                    ln2 = smp.tile([1, 512], F32, tag="row")
                    nc.scalar.activation(ln2, ss2, AF.Ln, bias=eps_t,
                                         scale=1.0 / D)
                    rr2 = smp.tile([1, 512], F32, tag="row")
                    nc.scalar.activation(rr2, ln2, AF.Exp, scale=-0.5)
                    bc2 = rbp.tile([P, 512], F32, tag="rb")
                    nc.gpsimd.partition_broadcast(bc2, rr2)
                    x1n = x1np_o.tile([P, NDT, 512], F32R, tag="x1n")
                    for dt_ in range(NDT):
                        nc.vector.tensor_mul(
                            x1n[:, dt_, :], x1T[:, dt_, qsl], bc2)
                    x1ns.append(x1n)
        # kqv closed: FFN pools reuse its SBUF space

        # ---- phase 3: FFN (f32r), quarter-of-hidden, both slices ----
        with ExitStack() as fctx:
            ghp = fctx.enter_context(tc.tile_pool(name="ghq", bufs=2))
            tsp = fctx.enter_context(tc.tile_pool(name="tsb", bufs=3))
            finp = fctx.enter_context(tc.tile_pool(name="fin", bufs=2))
            wghp = fctx.enter_context(tc.tile_pool(name="wgh", bufs=2))
            wobp = fctx.enter_context(tc.tile_pool(name="wob", bufs=2))
            wgr = d["wgT"].rearrange("(dt p) f -> p dt f", p=P)
            whr = d["whT"].rearrange("(dt p) f -> p dt f", p=P)
            wor8 = d["woB"].rearrange("(ht p) f -> p ht f", p=P)
            NQ = NHT // 4
            for ns in range(2):
                qsl = slice(ns * 512, (ns + 1) * 512)
                x1n = x1ns[ns]
                for q4 in range(4):
                    ghq = ghp.tile([P, NQ, 512], F32R, tag="ghq")
                    for hl in range(NQ):
                        ht = q4 * NQ + hl
                        wg = wghp.tile([P, NDT, P], F32R, tag="wg")
                        wh = wghp.tile([P, NDT, P], F32R, tag="wh")
                        nc.sync.dma_start(
                            out=wg, in_=wgr[:, :, ht * P:(ht + 1) * P])
                        nc.sync.dma_start(
                            out=wh, in_=whr[:, :, ht * P:(ht + 1) * P])
                        g_ps = gpsp.tile([P, 512], F32, tag="g")
                        h_ps = hpsp.tile([P, 512], F32, tag="h")
                        for dt_ in range(NDT):
                            nc.tensor.matmul(
                                g_ps, wg[:, dt_, :], x1n[:, dt_, :],
                                start=(dt_ == 0), stop=(dt_ == NDT - 1))
                        for dt_ in range(NDT):
                            nc.tensor.matmul(
                                h_ps, wh[:, dt_, :], x1n[:, dt_, :],
                                start=(dt_ == 0), stop=(dt_ == NDT - 1))
                        # silu(g)*h via tanh: t=tanh(g/2); gh=0.5*g*(1+t)*h
                        t_sb = tsp.tile([P, 512], F32, tag="tanh")
                        nc.scalar.activation(t_sb, g_ps, AF.Tanh, scale=0.5)
                        tmp = tsp.tile([P, 512], F32, tag="tmp")
                        nc.vector.scalar_tensor_tensor(
                            out=tmp, in0=t_sb, scalar=1.0, in1=g_ps,
                            op0=ALU.add, op1=ALU.mult)
                        nc.vector.scalar_tensor_tensor(
                            out=ghq[:, hl, :], in0=tmp, scalar=0.5,
                            in1=h_ps, op0=ALU.mult, op1=ALU.mult)
                    for fo in range(NDT):
                        wob = wobp.tile([P, NQ, P], F32R, tag="wob")
                        nc.sync.dma_start(
                            out=wob,
                            in_=wor8[:, q4 * NQ:(q4 + 1) * NQ,
                                     fo * P:(fo + 1) * P])
                        fp = mpsp.tile([P, 512], F32, tag="m")
                        for hl in range(NQ):
                            nc.tensor.matmul(
                                fp, wob[:, hl, :], ghq[:, hl, :],
                                start=(hl == 0), stop=(hl == NQ - 1))
                        if q4 < 3:
                            nc.vector.tensor_add(
                                x1T[:, fo, qsl], fp, x1T[:, fo, qsl])
                        else:
                            yt = finp.tile([P, 512], F32, tag="yt")
                            nc.vector.scalar_tensor_tensor(
                                out=yt, in0=fp,
                                scalar=bout_sb[:, fo:fo + 1],
                                in1=x1T[:, fo, qsl],
                                op0=ALU.add, op1=ALU.add)
                            nc.gpsimd.dma_start(
                                out=d["yT"][fo * P:(fo + 1) * P, qsl],
                                in_=yt)


_NC_CACHE = {}


def kernel(x, W_q, W_k, W_v, W_o, b_o, attn_norm_w, ffn_norm_w,
           W_gate, W_hidden, W_out, b_out):
    x = np.asarray(x, np.float32)
    f32 = lambda a: np.ascontiguousarray(np.asarray(a, np.float32))
    bf16 = lambda a: np.ascontiguousarray(
        np.asarray(a, np.float32).astype(ml_dtypes.bfloat16))
    w1 = np.asarray(attn_norm_w, np.float32)[:, None]
    w2 = np.asarray(ffn_norm_w, np.float32)[:, None]
    wqT = bf16(np.asarray(W_q, np.float32).T * w1 / np.sqrt(HD))
    wkT = bf16(np.asarray(W_k, np.float32).T * w1)
    wvT = bf16(np.asarray(W_v, np.float32).T * w1)
    woT = bf16(np.asarray(W_o, np.float32).T)
    def f22(a):
        b = np.ascontiguousarray(np.asarray(a, np.float32)).view(np.uint32)
        return ((b >> 10) << 10).view(np.float32)
    wgT = f22(np.asarray(W_gate, np.float32).T * w2)
    whT = f22(np.asarray(W_hidden, np.float32).T * w2)
    woB = f22(np.asarray(W_out, np.float32).T)
    bo_t = f32(np.asarray(b_o, np.float32).reshape(NDT, P).T)
    bout_t = f32(np.asarray(b_out, np.float32).reshape(NDT, P).T)

    if "nc" not in _NC_CACHE:
        _NC_CACHE["nc"] = build_nc()
    nc = _NC_CACHE["nc"]

    in_maps = []
    for c in range(8):
        b, half = c // 2, c % 2
        xb = x[b]
        if half:
            xb = np.concatenate([xb[LQ:], xb[:LQ]], axis=0)
        in_maps.append({
            "xT": np.ascontiguousarray(xb.T),
            "wqT": wqT, "wkT": wkT, "wvT": wvT, "woT": woT,
            "bo_t": bo_t, "wgT": wgT, "whT": whT, "woB": woB,
            "bout_t": bout_t,
        })
    global _LAST_IN_MAPS
    _LAST_IN_MAPS = in_maps
    res = bass_utils.run_bass_kernel_spmd(nc, in_maps, core_ids=list(range(8)))
    y = np.empty((4, L, D), np.float32)
    for c in range(8):
        b, half = c // 2, c % 2
        y[b, half * LQ:(half + 1) * LQ, :] = res.results[c]["yT"].T
    return y


# revision 29
# speedup vs baseline: 1.0089x; 1.0089x over previous
"""Dense transformer block (RMSNorm+MHA+residual, RMSNorm+SwiGLU+residual)
on 8 trn2 NeuronCores. Sharding: 2 cores per batch element; each core
computes the block output for 1024 of its batch's 2048 tokens, redundantly
computing K/V for the full sequence (attention keys are permutation
invariant, so each core's xT puts its own 1024 query tokens first).
No inter-core communication.

All on-chip tensors are feature-major ([feature, token]) so every matmul
contraction lands on the partition dim. Softmax denominators come free
from a ones-column appended to V.

v2: fp8 DoubleRow FFN (2x tensor throughput), softmax exp batched over
4 PSUM banks per ACTIVATE (amortizes the per-instruction overhead),
silu via tanh / rsqrt via ln+exp (keeps the scalar engine mostly on one
activation table set), gpsimd partition-broadcast instead of PSUM
broadcast matmuls, batched strided DMAs, and token-sliced emission so
FFN matmuls overlap the exp-bound attention window.
"""
import sys
from contextlib import ExitStack

import numpy as np

sys.path.insert(0, "/opt/trn_rl_repo")

import ml_dtypes  # noqa: E402
import concourse.bass as bass  # noqa: E402
from concourse import bacc  # noqa: E402
import concourse.tile as tile  # noqa: E402
from concourse import mybir  # noqa: E402
from concourse import bass_utils  # noqa: E402

P = 128
D = 1024          # d_model
L = 2048          # full seq per core (keys)
LQ = 1024         # query tokens per core
NH = 16
HD = 64
HID = 4096
EPS = 1e-6
NDT = D // P      # 8 feature tiles
NKT = L // P      # 16 key tiles
NHT = HID // P    # 32 hidden tiles
G = 2             # kt blocks per exp group (2 PSUM banks)
GROUPS = [(k, 2) for k in range(0, 16, 2)]
SW = 64.0         # fp8 weight scale (wg8/wh8/wo8 = 64*w)
SGH = 16.0        # ghT8 = 16 * silu(g) * h
F32 = mybir.dt.float32
BF16 = mybir.dt.bfloat16
FP8 = mybir.dt.float8e4
F32R = mybir.dt.float32r
AF = mybir.ActivationFunctionType
ALU = mybir.AluOpType
DR = mybir.MatmulPerfMode.DoubleRow

SIM_TIME_NS = None


def build_nc():
    global SIM_TIME_NS
    nc = bacc.Bacc(None, target_bir_lowering=False)
    d = {}
    d["xT"] = nc.dram_tensor("xT", [D, L], F32, kind="ExternalInput")
    d["wqT"] = nc.dram_tensor("wqT", [D, D], BF16, kind="ExternalInput")
    d["wkT"] = nc.dram_tensor("wkT", [D, D], BF16, kind="ExternalInput")
    d["wvT"] = nc.dram_tensor("wvT", [D, D], BF16, kind="ExternalInput")
    d["woT"] = nc.dram_tensor("woT", [D, D], BF16, kind="ExternalInput")
    d["bo_t"] = nc.dram_tensor("bo_t", [P, NDT], F32, kind="ExternalInput")
    d["wgT"] = nc.dram_tensor("wgT", [D, HID], F32R, kind="ExternalInput")
    d["whT"] = nc.dram_tensor("whT", [D, HID], F32R, kind="ExternalInput")
    d["woB"] = nc.dram_tensor("woB", [HID, D], F32R, kind="ExternalInput")
    d["bout_t"] = nc.dram_tensor("bout_t", [P, NDT], F32, kind="ExternalInput")
    d["yT"] = nc.dram_tensor("yT", [D, LQ], F32, kind="ExternalOutput")

    with tile.TileContext(nc) as tc:
        _body(tc, nc, d)
        _, snap = tc.schedule_and_allocate()
        SIM_TIME_NS = snap.time
    nc.compile()
    return nc


def _body(tc, nc, d):
    xTr = d["xT"].rearrange("(dt p) l -> p dt l", p=P)
    with ExitStack() as pp_ctx:
        pp = pp_ctx.enter_context(tc.tile_pool(name="persist", bufs=1))
        bo_sb = pp.tile([P, NDT], F32, tag="bo")
        bout_sb = pp.tile([P, NDT], F32, tag="bout")
        ones_col = pp.tile([P, 1], BF16, tag="ones")
        eps_t = pp.tile([1, 1], F32, tag="eps")
        x1T = pp.tile([P, NDT, LQ], F32, tag="x1T")
        x1np_o = pp_ctx.enter_context(tc.tile_pool(name="x1n", bufs=2))
        # gate/hidden/misc PSUM pools span attention (Wo/ss2) and FFN
        gpsp = pp_ctx.enter_context(tc.tile_pool(name="gps", bufs=1, space="PSUM"))
        hpsp = pp_ctx.enter_context(tc.tile_pool(name="hps", bufs=1, space="PSUM"))
        mpsp = pp_ctx.enter_context(tc.tile_pool(name="mps", bufs=1, space="PSUM"))
        nc.sync.dma_start(out=bo_sb, in_=d["bo_t"][:, :])
        nc.sync.dma_start(out=bout_sb, in_=d["bout_t"][:, :])
        nc.vector.memset(ones_col, 1.0)
        nc.vector.memset(eps_t, EPS)
        x1ns = []

        # kqv pool closes after attention so FFN pools reuse its SBUF
        with tc.tile_pool(name="kqv", bufs=1) as kqvp:
            kT = kqvp.tile([P, NDT, L], BF16, tag="kT")
            qT = kqvp.tile([P, NDT, LQ], BF16, tag="qT")
            vt = kqvp.tile([P, NKT, NH, HD + 1], BF16, tag="vt")
            nc.vector.memset(vt[:, :, :, HD:HD + 1], 1.0)

            # ---- phase 0/1: load x, rmsnorm in place, project K/Q/V ----
            with tc.tile_pool(name="xt", bufs=1) as xtp:
                xt = xtp.tile([P, NDT, L], BF16, tag="xt")
                for ls in range(L // 512):
                    sl = slice(ls * 512, (ls + 1) * 512)
                    nc.gpsimd.dma_start(out=xt[:, :, sl], in_=xTr[:, :, sl])
                with tc.tile_pool(name="n1", bufs=3) as n1p, \
                     tc.tile_pool(name="bc1", bufs=1) as bc1p, \
                     tc.tile_pool(name="ss1", bufs=1, space="PSUM") as ssp:
                    ss_ps = ssp.tile([1, L], F32, tag="ss")
                    bc = bc1p.tile([P, L], F32, tag="bc1")
                    for ls in range(L // 512):
                        sl = slice(ls * 512, (ls + 1) * 512)
                        for dt_ in range(NDT):
                            sq = n1p.tile([P, 512], BF16, tag="sq")
                            nc.vector.tensor_mul(sq, xt[:, dt_, sl],
                                                 xt[:, dt_, sl])
                            nc.tensor.matmul(ss_ps[:, sl], ones_col, sq,
                                             start=(dt_ == 0),
                                             stop=(dt_ == NDT - 1))
                        lnr = bc1p.tile([1, 512], F32, tag="lnr")
                        nc.scalar.activation(lnr, ss_ps[:, sl], AF.Ln,
                                             bias=eps_t, scale=1.0 / D)
                        rr = bc1p.tile([1, 512], F32, tag="rr1")
                        nc.scalar.activation(rr, lnr, AF.Exp, scale=-0.5)
                        nc.gpsimd.partition_broadcast(bc[:, sl], rr)
                        for dt_ in range(NDT):
                            nc.vector.tensor_mul(
                                xt[:, dt_, sl], xt[:, dt_, sl], bc[:, sl])

                # K, Q projections (K first: scores depend on it)
                with tc.tile_pool(name="wblk", bufs=2) as wp, \
                     tc.tile_pool(name="wv", bufs=1) as wvp, \
                     tc.tile_pool(name="proj", bufs=4, space="PSUM") as prp:
                    for (w_d, dst, ntok) in ((d["wkT"], kT, L),
                                             (d["wqT"], qT, LQ)):
                        wr = w_d.rearrange("(dt p) f -> p dt f", p=P)
                        for ft in range(NDT):
                            wblk = wp.tile([P, NDT, P], BF16, tag="wblk")
                            nc.sync.dma_start(
                                out=wblk, in_=wr[:, :, ft * P:(ft + 1) * P])
                            for ns in range(ntok // 512):
                                sl = slice(ns * 512, (ns + 1) * 512)
                                ps = prp.tile([P, 512], F32, tag="pp")
                                for dt_ in range(NDT):
                                    nc.tensor.matmul(
                                        ps, wblk[:, dt_, :], xt[:, dt_, sl],
                                        start=(dt_ == 0),
                                        stop=(dt_ == NDT - 1))
                                nc.scalar.activation(dst[:, ft, sl], ps,
                                                     AF.Copy)
                    wvr = d["wvT"].rearrange("(dt p) f -> p dt f", p=P)
                    for hf in range(2):
                        wv = wvp.tile([P, NDT, 512], BF16, tag="wv")
                        nc.sync.dma_start(
                            out=wv, in_=wvr[:, :, hf * 512:(hf + 1) * 512])
                        for tt in range(NKT):
                            ps = prp.tile([P, 512], F32, tag="pp")
                            for dt_ in range(NDT):
                                nc.tensor.matmul(
                                    ps, xt[:, dt_, tt * P:(tt + 1) * P],
                                    wv[:, dt_, :],
                                    start=(dt_ == 0), stop=(dt_ == NDT - 1))
                            nc.vector.tensor_copy(
                                vt[:, tt, hf * 8:(hf + 1) * 8, 0:HD],
                                ps.rearrange("p (h e) -> p h e", h=8))

            # ---- phase 2: attention + Wo + norm2 per 512-token slice ----
            with ExitStack() as ectx:
                atp = ectx.enter_context(tc.tile_pool(name="attnp", bufs=1))
                ptp = ectx.enter_context(tc.tile_pool(name="pt", bufs=3))
                smp = ectx.enter_context(tc.tile_pool(name="sm", bufs=2))
                rbp = ectx.enter_context(tc.tile_pool(name="rb", bufs=2))
                xqp = ectx.enter_context(tc.tile_pool(name="xq", bufs=3))
                sqp = ectx.enter_context(tc.tile_pool(name="sq2p", bufs=2))
                wop = ectx.enter_context(tc.tile_pool(name="wo", bufs=3))
                stp = ectx.enter_context(
                    tc.tile_pool(name="st", bufs=2, space="PSUM"))
                accp = ectx.enter_context(
                    tc.tile_pool(name="acc", bufs=1, space="PSUM"))
                for ns in range(2):
                    qsl = slice(ns * 512, (ns + 1) * 512)
                    attnT = atp.tile([P, NDT, 512], BF16, tag="attnT")
                    for h in range(NH):
                        dt_ = h // 2
                        r0 = (h % 2) * HD
                        acc = accp.tile([HD + 1, 512], F32, tag="acc")
                        for (k0, glen) in GROUPS:
                            st = stp.tile([P, glen, 512], F32, tag="st")
                            for j in range(glen):
                                kt = k0 + j
                                nc.tensor.matmul(
                                    st[:, j, :],
                                    kT[r0:r0 + HD, dt_, kt * P:(kt + 1) * P],
                                    qT[r0:r0 + HD, dt_, qsl],
                                    start=True, stop=True)
                            pt = ptp.tile([P, glen, 512], BF16, tag="pt")
                            nc.scalar.activation(pt, st, AF.Exp)
                            for j in range(glen):
                                kt = k0 + j
                                nc.tensor.matmul(
                                    acc, vt[:, kt, h, :], pt[:, j, :],
                                    start=(kt == 0), stop=(kt == NKT - 1))
                        accS = smp.tile([HD + 1, 512], F32, tag="accS")
                        nc.vector.tensor_copy(accS, acc)
                        rrow = smp.tile([1, 512], F32, tag="row")
                        nc.vector.reciprocal(rrow, accS[HD:HD + 1, :])
                        rb = rbp.tile([HD, 512], F32, tag="rb")
                        nc.gpsimd.partition_broadcast(rb, rrow)
                        nc.vector.tensor_mul(
                            attnT[r0:r0 + HD, dt_, :], accS[0:HD, :], rb)

                    # Wo projection + residual -> x1T
                    wor = d["woT"].rearrange("(dt p) f -> p dt f", p=P)
                    for ft in range(NDT):
                        wblk = wop.tile([P, NDT, P], BF16, tag="woblk")
                        nc.sync.dma_start(
                            out=wblk, in_=wor[:, :, ft * P:(ft + 1) * P])
                        xq = xqp.tile([P, 512], F32, tag="xq")
                        nc.gpsimd.dma_start(out=xq, in_=xTr[:, ft, qsl])
                        ps = mpsp.tile([P, 512], F32, tag="m")
                        for fi in range(NDT):
                            nc.tensor.matmul(
                                ps, wblk[:, fi, :], attnT[:, fi, :],
                                start=(fi == 0), stop=(fi == NDT - 1))
                        nc.vector.scalar_tensor_tensor(
                            out=x1T[:, ft, qsl], in0=ps,
                            scalar=bo_sb[:, ft:ft + 1],
                            in1=xq, op0=ALU.add, op1=ALU.add)

                    # rmsnorm2 -> x1n (f32r for the f22 FFN)
                    ss2 = mpsp.tile([1, 512], F32, tag="m")
                    for dt_ in range(NDT):
                        sq2 = sqp.tile([P, 512], BF16, tag="sq2")
                        nc.vector.tensor_mul(
                            sq2, x1T[:, dt_, qsl], x1T[:, dt_, qsl])
                        nc.tensor.matmul(ss2, ones_col, sq2,
                                         start=(dt_ == 0),
                                         stop=(dt_ == NDT - 1))
                    ln2 = smp.tile([1, 512], F32, tag="row")
                    nc.scalar.activation(ln2, ss2, AF.Ln, bias=eps_t,
                                         scale=1.0 / D)
                    rr2 = smp.tile([1, 512], F32, tag="row")
                    nc.scalar.activation(rr2, ln2, AF.Exp, scale=-0.5)
                    bc2 = rbp.tile([P, 512], F32, tag="rb")
                    nc.gpsimd.partition_broadcast(bc2, rr2)
                    x1n = x1np_o.tile([P, NDT, 512], F32R, tag="x1n")
                    for dt_ in range(NDT):
                        nc.vector.tensor_mul(
                            x1n[:, dt_, :], x1T[:, dt_, qsl], bc2)
                    x1ns.append(x1n)
        # kqv closed: FFN pools reuse its SBUF space

        # ---- phase 3: FFN (f32r), quarter-of-hidden, both slices ----
        with ExitStack() as fctx:
            fpp = fctx.enter_context(
                tc.tile_pool(name="fpp", bufs=2, space="PSUM"))
            gpsp2 = fctx.enter_context(
                tc.tile_pool(name="gps2", bufs=1, space="PSUM"))
            hpsp2 = fctx.enter_context(
                tc.tile_pool(name="hps2", bufs=1, space="PSUM"))
            ghp = fctx.enter_context(tc.tile_pool(name="ghq", bufs=2))
            tsp = fctx.enter_context(tc.tile_pool(name="tsb", bufs=3))
            finp = fctx.enter_context(tc.tile_pool(name="fin", bufs=2))
            wghp = fctx.enter_context(tc.tile_pool(name="wgh", bufs=2))
            wobp = fctx.enter_context(tc.tile_pool(name="wob", bufs=2))
            wgr = d["wgT"].rearrange("(dt p) f -> p dt f", p=P)
            whr = d["whT"].rearrange("(dt p) f -> p dt f", p=P)
            wor8 = d["woB"].rearrange("(ht p) f -> p ht f", p=P)
            NQ = NHT // 4
            for ns in range(2):
                qsl = slice(ns * 512, (ns + 1) * 512)
                x1n = x1ns[ns]
                for q4 in range(4):
                    ghq = ghp.tile([P, NQ, 512], F32R, tag="ghq")
                    for hl in range(NQ):
                        ht = q4 * NQ + hl
                        wg = wghp.tile([P, NDT, P], F32R, tag="wg")
                        wh = wghp.tile([P, NDT, P], F32R, tag="wh")
                        nc.sync.dma_start(
                            out=wg, in_=wgr[:, :, ht * P:(ht + 1) * P])
                        nc.sync.dma_start(
                            out=wh, in_=whr[:, :, ht * P:(ht + 1) * P])
                        if ht % 2 == 0:
                            g_ps = gpsp.tile([P, 512], F32, tag="g")
                            h_ps = hpsp.tile([P, 512], F32, tag="h")
                        else:
                            g_ps = gpsp2.tile([P, 512], F32, tag="g2")
                            h_ps = hpsp2.tile([P, 512], F32, tag="h2")
                        for dt_ in range(NDT):
                            nc.tensor.matmul(
                                g_ps, wg[:, dt_, :], x1n[:, dt_, :],
                                start=(dt_ == 0), stop=(dt_ == NDT - 1))
                        for dt_ in range(NDT):
                            nc.tensor.matmul(
                                h_ps, wh[:, dt_, :], x1n[:, dt_, :],
                                start=(dt_ == 0), stop=(dt_ == NDT - 1))
                        # silu(g)*h via tanh: t=tanh(g/2); gh=0.5*g*(1+t)*h
                        t_sb = tsp.tile([P, 512], F32, tag="tanh")
                        nc.scalar.activation(t_sb, g_ps, AF.Tanh, scale=0.5)
                        tmp = tsp.tile([P, 512], F32, tag="tmp")
                        nc.vector.scalar_tensor_tensor(
                            out=tmp, in0=t_sb, scalar=1.0, in1=g_ps,
                            op0=ALU.add, op1=ALU.mult)
                        nc.vector.scalar_tensor_tensor(
                            out=ghq[:, hl, :], in0=tmp, scalar=0.5,
                            in1=h_ps, op0=ALU.mult, op1=ALU.mult)
                    for fo in range(NDT):
                        wob = wobp.tile([P, NQ, P], F32R, tag="wob")
                        nc.sync.dma_start(
                            out=wob,
                            in_=wor8[:, q4 * NQ:(q4 + 1) * NQ,
                                     fo * P:(fo + 1) * P])
                        fp = fpp.tile([P, 512], F32, tag="fp")
                        for hl in range(NQ):
                            nc.tensor.matmul(
                                fp, wob[:, hl, :], ghq[:, hl, :],
                                start=(hl == 0), stop=(hl == NQ - 1))
                        if q4 < 3:
                            nc.vector.tensor_add(
                                x1T[:, fo, qsl], fp, x1T[:, fo, qsl])
                        else:
                            yt = finp.tile([P, 512], F32, tag="yt")
                            nc.vector.scalar_tensor_tensor(
                                out=yt, in0=fp,
                                scalar=bout_sb[:, fo:fo + 1],
                                in1=x1T[:, fo, qsl],
                                op0=ALU.add, op1=ALU.add)
                            nc.gpsimd.dma_start(
                                out=d["yT"][fo * P:(fo + 1) * P, qsl],
                                in_=yt)


_NC_CACHE = {}


def kernel(x, W_q, W_k, W_v, W_o, b_o, attn_norm_w, ffn_norm_w,
           W_gate, W_hidden, W_out, b_out):
    x = np.asarray(x, np.float32)
    f32 = lambda a: np.ascontiguousarray(np.asarray(a, np.float32))
    bf16 = lambda a: np.ascontiguousarray(
        np.asarray(a, np.float32).astype(ml_dtypes.bfloat16))
    w1 = np.asarray(attn_norm_w, np.float32)[:, None]
    w2 = np.asarray(ffn_norm_w, np.float32)[:, None]
    wqT = bf16(np.asarray(W_q, np.float32).T * w1 / np.sqrt(HD))
    wkT = bf16(np.asarray(W_k, np.float32).T * w1)
    wvT = bf16(np.asarray(W_v, np.float32).T * w1)
    woT = bf16(np.asarray(W_o, np.float32).T)
    def f22(a):
        b = np.ascontiguousarray(np.asarray(a, np.float32)).view(np.uint32)
        return ((b >> 10) << 10).view(np.float32)
    wgT = f22(np.asarray(W_gate, np.float32).T * w2)
    whT = f22(np.asarray(W_hidden, np.float32).T * w2)
    woB = f22(np.asarray(W_out, np.float32).T)
    bo_t = f32(np.asarray(b_o, np.float32).reshape(NDT, P).T)
    bout_t = f32(np.asarray(b_out, np.float32).reshape(NDT, P).T)

    if "nc" not in _NC_CACHE:
        _NC_CACHE["nc"] = build_nc()
    nc = _NC_CACHE["nc"]

    in_maps = []
    for c in range(8):
        b, half = c // 2, c % 2
        xb = x[b]
        if half:
            xb = np.concatenate([xb[LQ:], xb[:LQ]], axis=0)
        in_maps.append({
            "xT": np.ascontiguousarray(xb.T),
            "wqT": wqT, "wkT": wkT, "wvT": wvT, "woT": woT,
            "bo_t": bo_t, "wgT": wgT, "whT": whT, "woB": woB,
            "bout_t": bout_t,
        })
    global _LAST_IN_MAPS
    _LAST_IN_MAPS = in_maps
    res = bass_utils.run_bass_kernel_spmd(nc, in_maps, core_ids=list(range(8)))
    y = np.empty((4, L, D), np.float32)
    for c in range(8):
        b, half = c // 2, c % 2
        y[b, half * LQ:(half + 1) * LQ, :] = res.results[c]["yT"].T
    return y
